# revision 2
# baseline (speedup 1.0000x reference)
"""Trainium2 Bass kernel for the constrained-CKY hinge loss problem.

Reference computation (fp32):
  - Two max-plus CKY DPs over a triangular chart (LENGTH=128, BATCH=256):
    one from a zero chart, one from a chart with +1000 bonuses at 8
    constraint cells per example.
  - Loss = masked mean of hinge(MARGIN + pred - constr).

Sharding: pure data parallel over (chart-type x batch-quarter):
  core c in 0..7 runs chart type c//4 (0=plain, 1=constrained) for batch
  slice (c%4)*64 : (c%4+1)*64.  64 batch rows live on 64 SBUF partitions;
  the whole DP for a row runs on its partition.

Chart layout per row: C[row][pos*128 + w] = cell(width=w, start=pos).
  At level l (L=128-l positions, N=l split points):
    ls[pos,n] = C[pos*128 + n]                  (strides pos:128, n:1)
    rs[pos,n] = C[(pos+n+1)*128 + (l-1-n)]
              = C[pos*128 + n*127 + (127+l)]    (strides pos:128, n:127)
  best[pos] = max_n(ls + rs + x);  C[pos*128 + l] = best + init_col.
"""

import sys

if "/opt/trn_rl_repo" not in sys.path:
    sys.path.insert(0, "/opt/trn_rl_repo")

import numpy as np

LENGTH = 128
BATCH = 256
MARGIN = 1.0
BONUS = 1000.0
NCELLS = LENGTH * (LENGTH + 1) // 2  # 8256
NCONSTR = 8
N_CORES = 8
ROWS = 64  # batch rows per core
CSTRIDE = 128  # free-dim stride between consecutive `pos` in the chart tile


def _offsets():
    off = np.zeros(LENGTH, dtype=np.int64)
    for lvl in range(1, LENGTH):
        off[lvl] = off[lvl - 1] + (LENGTH - (lvl - 1))
    return off


OFF = _offsets()


def _score_offsets():
    soff, acc = {}, 0
    for level in range(1, LENGTH):
        soff[level] = acc
        acc += (LENGTH - level) * level
    return soff, acc


SOFF, TOTAL = _score_offsets()  # TOTAL = 349504

_COMPILED = None


def _build_program(repeat=1):
    from concourse import bacc, bass, mybir
    from concourse import tile

    nc = bacc.Bacc("TRN2", target_bir_lowering=False, debug=False,
                   num_devices=N_CORES)
    scores_ext = nc.dram_tensor("scores", [ROWS, TOTAL], mybir.dt.float32,
                                kind="ExternalInput")
    roots_ext = nc.dram_tensor("roots", [ROWS, 1], mybir.dt.float32,
                               kind="ExternalOutput")

    f32 = mybir.dt.float32
    ADD = mybir.AluOpType.add
    MAX = mybir.AluOpType.max

    with tile.TileContext(nc) as tc:
        with (
            tc.tile_pool(name="persist", bufs=1) as persist,
            tc.tile_pool(name="xs", bufs=6) as xpool,
            tc.tile_pool(name="tmp", bufs=1) as tpool,
            tc.tile_pool(name="tmp2", bufs=1) as upool,
        ):
            C = persist.tile([ROWS, LENGTH * CSTRIDE], f32)

            c_full = C[:]
            c_part = c_full.ap[0]  # [partition_stride, ROWS]

            def c_view(offset, dims):
                return bass.AP(tensor=c_full.tensor,
                               offset=c_full.offset + offset,
                               ap=[c_part] + dims)

            for _rep in range(repeat):
                # level 0 = 0 everywhere (bonuses are folded into scores
                # on the host; constraint cells always have width >= 1).
                nc.vector.memset(C[:], 0.0)

                for l in range(1, LENGTH):
                    L = LENGTH - l
                    N = l
                    E = L * N

                    x_t = xpool.tile([ROWS, E], f32, tag="x")
                    nc.sync.dma_start(out=x_t[:],
                                      in_=scores_ext[:, SOFF[l]:SOFF[l] + E])

                    ls = c_view(0, [[CSTRIDE, L], [1, N]])
                    rs = c_view(127 + l, [[CSTRIDE, L], [127, N]])

                    t_t = tpool.tile([ROWS, E], f32, tag="t")
                    tf = t_t[:]
                    t3 = bass.AP(tensor=tf.tensor, offset=tf.offset,
                                 ap=[tf.ap[0], [N, L], [1, N]])
                    nc.vector.tensor_tensor(out=t3, in0=ls, in1=rs, op=ADD)

                    u_t = upool.tile([ROWS, E], f32, tag="u")
                    nc.vector.tensor_tensor(out=u_t[:], in0=t_t[:],
                                            in1=x_t[:], op=ADD)
                    uf = u_t[:]
                    u3 = bass.AP(tensor=uf.tensor, offset=uf.offset,
                                 ap=[uf.ap[0], [N, L], [1, N]])

                    # best[pos] written straight into the chart column
                    nc.vector.tensor_reduce(out=c_view(l, [[CSTRIDE, L]]),
                                            in_=u3,
                                            axis=mybir.AxisListType.X, op=MAX)

            # root = cell(width=127, pos=0) = C[127]
            nc.sync.dma_start(out=roots_ext[:], in_=C[:, 127:128])

    nc.compile()
    return nc


def _get_compiled():
    global _COMPILED
    if _COMPILED is None:
        _COMPILED = _build_program()
    return _COMPILED


def build_inmaps(scores, constraint_pos):
    """Per-core input maps: cores 0-3 raw scores (plain chart), cores 4-7
    bonus-folded scores (constrained chart), batch quartered."""
    scores = np.asarray(scores, dtype=np.float32)
    constraint_pos = np.asarray(constraint_pos, dtype=np.int32)
    B = scores.shape[0]
    assert B == BATCH and scores.shape[1] == TOTAL

    # Constrained-chart scores: fold the +BONUS of cell (w, p) into every
    # split entry x[l=w][p, n] of that cell (exact up to 1 ulp vs adding it
    # after the reduce).  Set semantics (duplicate constraints count once),
    # matching `chart.at[rows, pos].set(BONUS)` in the reference.
    folded = scores.copy()
    bonus = np.float32(BONUS)
    for b in range(B):
        for f in set(int(v) for v in constraint_pos[b]):
            w = int(np.searchsorted(OFF, f, side="right")) - 1
            p = f - int(OFF[w])
            s = SOFF[w] + p * w
            folded[b, s:s + w] += bonus

    in_maps = []
    for c in range(N_CORES):
        q = c % 4
        sl = slice(q * ROWS, (q + 1) * ROWS)
        src = scores if c < 4 else folded
        in_maps.append({"scores": np.ascontiguousarray(src[sl])})
    return in_maps


def kernel(scores, constraint_pos, trace=False):
    from concourse.bass_utils import run_bass_kernel_spmd

    in_maps = build_inmaps(scores, constraint_pos)

    nc = _get_compiled()
    res = run_bass_kernel_spmd(nc, in_maps, list(range(N_CORES)), trace=trace)

    pred = np.concatenate([res.results[q]["roots"][:, 0] for q in range(4)])
    constr_root = np.concatenate(
        [res.results[4 + q]["roots"][:, 0] for q in range(4)])

    pred = pred.astype(np.float32)
    constr = (constr_root - np.float32(BONUS * NCONSTR)).astype(np.float32)

    mask = (np.abs(pred - constr) >= np.float32(0.001)).astype(np.float32)
    hinge = np.maximum(np.float32(MARGIN) + pred - constr,
                       np.float32(0.0)) * mask
    msum = mask.sum(dtype=np.float32)
    hsum = hinge.sum(dtype=np.float32)
    if msum > np.float32(0.1):
        out = hsum / np.maximum(msum, np.float32(1.0))
    else:
        out = hsum
    result = np.asarray(out, dtype=np.float32)
    if trace:
        return result, res
    return result



# revision 4
# speedup vs baseline: 7.8980x; 7.8980x over previous
"""Trainium2 Bass kernel for the constrained-CKY hinge loss problem.

Reference computation (fp32):
  - Two max-plus CKY DPs over a triangular chart (LENGTH=128, BATCH=256):
    one from a zero chart, one from a chart with +1000 bonuses at 8
    constraint cells per example.
  - Loss = masked mean of hinge(MARGIN + pred - constr).

Sharding: pure data parallel over (chart-type x batch-quarter):
  core c in 0..7 runs chart type c//4 (0=plain, 1=constrained) for batch
  slice (c%4)*64 : (c%4+1)*64.  64 batch rows live on 64 SBUF partitions;
  the whole DP for a row runs on its partition.

Chart layout per row: C[row][pos*128 + w] = cell(width=w, start=pos).
  At level l (L=128-l positions, N=l split points):
    ls[pos,n] = C[pos*128 + n]                  (strides pos:128, n:1)
    rs[pos,n] = C[(pos+n+1)*128 + (l-1-n)]
              = C[pos*128 + n*127 + (127+l)]    (strides pos:128, n:127)
  best[pos] = max_n(ls + rs + x);  C[pos*128 + l] = best + init_col.
"""

import sys

if "/opt/trn_rl_repo" not in sys.path:
    sys.path.insert(0, "/opt/trn_rl_repo")

import numpy as np

LENGTH = 128
BATCH = 256
MARGIN = 1.0
BONUS = 1000.0
NCELLS = LENGTH * (LENGTH + 1) // 2  # 8256
NCONSTR = 8
N_CORES = 8
ROWS = 64  # batch rows per core
CSTRIDE = 128  # free-dim stride between consecutive `pos` in the chart tile


def _offsets():
    off = np.zeros(LENGTH, dtype=np.int64)
    for lvl in range(1, LENGTH):
        off[lvl] = off[lvl - 1] + (LENGTH - (lvl - 1))
    return off


OFF = _offsets()


def _score_offsets():
    soff, acc = {}, 0
    for level in range(1, LENGTH):
        soff[level] = acc
        acc += (LENGTH - level) * level
    return soff, acc


SOFF, TOTAL = _score_offsets()  # TOTAL = 349504

_COMPILED = None

# DVE takes n in [0, ns), Pool engine (gpsimd) takes [ns, N).
# Balance: DVE 1.0417 ns/elem vs Pool 1.984 ns/elem on tensor_tensor.
POOL_ALPHA = 0.6557
POOL_MIN_E = 1024  # below this, pool instruction overhead beats the offload


def _build_program(repeat=1):
    from concourse import bacc, bass, mybir
    from concourse import tile

    nc = bacc.Bacc("TRN2", target_bir_lowering=False, debug=False,
                   num_devices=N_CORES)
    scores_ext = nc.dram_tensor("scores", [ROWS, TOTAL], mybir.dt.float32,
                                kind="ExternalInput")
    roots_ext = nc.dram_tensor("roots", [ROWS, 1], mybir.dt.float32,
                               kind="ExternalOutput")

    f32 = mybir.dt.float32
    ADD = mybir.AluOpType.add
    MAX = mybir.AluOpType.max

    with tile.TileContext(nc) as tc:
        with (
            tc.tile_pool(name="persist", bufs=1) as persist,
            tc.tile_pool(name="xs", bufs=6) as xpool,
            tc.tile_pool(name="tmp", bufs=1) as tpool,
            tc.tile_pool(name="tmp2", bufs=1) as upool,
        ):
            C = persist.tile([ROWS, LENGTH * CSTRIDE], f32)

            c_full = C[:]
            c_part = c_full.ap[0]  # [partition_stride, ROWS]

            def c_view(offset, dims):
                return bass.AP(tensor=c_full.tensor,
                               offset=c_full.offset + offset,
                               ap=[c_part] + dims)

            def sub3(tile_ap, N, L, a, b):
                """[(pos, n)] chunk view of a flat [ROWS, L*N] tile,
                n in [a, b)."""
                return bass.AP(tensor=tile_ap.tensor,
                               offset=tile_ap.offset + a,
                               ap=[tile_ap.ap[0], [N, L], [1, b - a]])

            for _rep in range(repeat):
                # Only width-0 cells (column 0) are read without being
                # written first; every other column w is written at level
                # w before any level > w reads it.  Bonuses are folded
                # into scores on the host (constraints have width >= 1).
                nc.vector.memset(c_view(0, [[CSTRIDE, LENGTH]]), 0.0)

                for l in range(1, LENGTH):
                    L = LENGTH - l
                    N = l
                    E = L * N

                    x_t = xpool.tile([ROWS, E], f32, tag="x")
                    nc.sync.dma_start(out=x_t[:],
                                      in_=scores_ext[:, SOFF[l]:SOFF[l] + E])

                    if E >= POOL_MIN_E and N >= 2:
                        ns = max(1, min(N - 1, int(round(POOL_ALPHA * N))))
                    else:
                        ns = N

                    t_t = tpool.tile([ROWS, E], f32, tag="t")
                    u_t = upool.tile([ROWS, E], f32, tag="u")
                    tf = t_t[:]
                    uf = u_t[:]
                    xf = x_t[:]

                    # ls/rs chunk views over the chart; t/u/x chunk views
                    # over the flat level tiles.
                    def ls_v(a, b):
                        return c_view(a, [[CSTRIDE, L], [1, b - a]])

                    def rs_v(a, b):
                        return c_view(127 + l + 127 * a,
                                      [[CSTRIDE, L], [127, b - a]])

                    # t = ls + rs ; u = t + x  (n-chunked across engines)
                    nc.vector.tensor_tensor(out=sub3(tf, N, L, 0, ns),
                                            in0=ls_v(0, ns), in1=rs_v(0, ns),
                                            op=ADD)
                    if ns < N:
                        nc.gpsimd.tensor_tensor(out=sub3(tf, N, L, ns, N),
                                                in0=ls_v(ns, N),
                                                in1=rs_v(ns, N), op=ADD)
                    nc.vector.tensor_tensor(out=sub3(uf, N, L, 0, ns),
                                            in0=sub3(tf, N, L, 0, ns),
                                            in1=sub3(xf, N, L, 0, ns),
                                            op=ADD)
                    if ns < N:
                        nc.gpsimd.tensor_tensor(out=sub3(uf, N, L, ns, N),
                                                in0=sub3(tf, N, L, ns, N),
                                                in1=sub3(xf, N, L, ns, N),
                                                op=ADD)

                    u3 = bass.AP(tensor=uf.tensor, offset=uf.offset,
                                 ap=[uf.ap[0], [N, L], [1, N]])

                    # best[pos] written straight into the chart column
                    nc.vector.tensor_reduce(out=c_view(l, [[CSTRIDE, L]]),
                                            in_=u3,
                                            axis=mybir.AxisListType.X, op=MAX)

            # root = cell(width=127, pos=0) = C[127]
            nc.sync.dma_start(out=roots_ext[:], in_=C[:, 127:128])

    nc.compile()
    return nc


def _get_compiled():
    global _COMPILED
    if _COMPILED is None:
        _COMPILED = _build_program()
    return _COMPILED


def build_inmaps(scores, constraint_pos):
    """Per-core input maps: cores 0-3 raw scores (plain chart), cores 4-7
    bonus-folded scores (constrained chart), batch quartered."""
    scores = np.asarray(scores, dtype=np.float32)
    constraint_pos = np.asarray(constraint_pos, dtype=np.int32)
    B = scores.shape[0]
    assert B == BATCH and scores.shape[1] == TOTAL

    # Constrained-chart scores: fold the +BONUS of cell (w, p) into every
    # split entry x[l=w][p, n] of that cell (exact up to 1 ulp vs adding it
    # after the reduce).  Set semantics (duplicate constraints count once),
    # matching `chart.at[rows, pos].set(BONUS)` in the reference.
    folded = scores.copy()
    bonus = np.float32(BONUS)
    for b in range(B):
        for f in set(int(v) for v in constraint_pos[b]):
            w = int(np.searchsorted(OFF, f, side="right")) - 1
            p = f - int(OFF[w])
            s = SOFF[w] + p * w
            folded[b, s:s + w] += bonus

    in_maps = []
    for c in range(N_CORES):
        q = c % 4
        sl = slice(q * ROWS, (q + 1) * ROWS)
        src = scores if c < 4 else folded
        in_maps.append({"scores": np.ascontiguousarray(src[sl])})
    return in_maps


def kernel(scores, constraint_pos, trace=False):
    from concourse.bass_utils import run_bass_kernel_spmd

    in_maps = build_inmaps(scores, constraint_pos)

    nc = _get_compiled()
    res = run_bass_kernel_spmd(nc, in_maps, list(range(N_CORES)), trace=trace)

    pred = np.concatenate([res.results[q]["roots"][:, 0] for q in range(4)])
    constr_root = np.concatenate(
        [res.results[4 + q]["roots"][:, 0] for q in range(4)])

    pred = pred.astype(np.float32)
    constr = (constr_root - np.float32(BONUS * NCONSTR)).astype(np.float32)

    mask = (np.abs(pred - constr) >= np.float32(0.001)).astype(np.float32)
    hinge = np.maximum(np.float32(MARGIN) + pred - constr,
                       np.float32(0.0)) * mask
    msum = mask.sum(dtype=np.float32)
    hsum = hinge.sum(dtype=np.float32)
    if msum > np.float32(0.1):
        out = hsum / np.maximum(msum, np.float32(1.0))
    else:
        out = hsum
    result = np.asarray(out, dtype=np.float32)
    if trace:
        return result, res
    return result

